# revision 46
# baseline (speedup 1.0000x reference)
"""AttentionPooling (segment softmax-pool) Trainium2 kernel, v3.

out[s,:] = sum_n 1[idx[n]==s] * gnorm[n] * (x[n,:] @ msg_w + msg_b)
  gnorm[n] = w[n]^p * exp(gate[n]) / (denom[seg] + eps)   (max-sub skipped:
  mathematically identical after normalization, logits are O(5))

v3 restructure vs the f32 baseline (647us -> target <250us):
  * everything bf16: PE matmuls 1 cyc/row (vs 4 for f32), DVE 2x/4x modes,
    half the HBM traffic.  rel tolerance is 2e-2; bf16 lands ~1e-3.
  * gate_w folded into x on the host: device sees xg = x * gate_w and
    msg_w' = msg_w / gate_w (exact identity: A@msg_w == (A*gw)@(msg_w/gw),
    denominator column unscaled).  The per-row gate logit then needs NO
    multiply on device -- just a reduction, done as a 7-level binary add
    tree (tensor_tensor has a 2x mode; tensor_reduce has none).  Levels
    1-2 are bf16 (the bulk of the work), levels 3-7 f32: an all-bf16
    tree costs 1.1e-2 rel err (vs the 2e-2 gate), this mix 4.6e-3.
  * one-hot G built per WINDOW (not per tile): host supplies a u8 one-hot
    mask strip [128, T*64]; one DVE tensor_tensor multiply against a
    stride-0 broadcast of gex[p,t] builds the whole window's G strip.
    (per-tile tensor_scalar builds cost ~207ns/op of fixed overhead;
    per-window strips amortize it 30x.)
  * exact per-window tiling: module is specialized (and cached) to the
    actual per-window tile counts (max over cores per window slot),
    ~1019 tiles/core vs 1088 padded.

Phase 1 per 128-row tile t of window w (64 segments per window):
  G[p,s]    = (iota[s] == idxl[p,t]) * gex[p,t]          (DVE, 4x)
  psA[s,c] += sum_p G[p,s] * xg[p,t,c]                   (PE, c = 0..128,
              col 128 is the ones column -> denominators)
Phase 2 per window: numer = (psA[:,0:128])^T-major matmul with msg_w',
  out = numer * (1/(denom+eps)) (+ msg_b term only if msg_b != 0).
"""

import os
import sys
import numpy as np

for _p in ("/opt/trn_rl_repo", "/root/.axon_site/_ro/trn_rl_repo"):
    if os.path.isdir(_p) and _p not in sys.path:
        sys.path.insert(0, _p)

P = 128
S = 16384
D = 128
NCORES = 8
WIN = 64                       # segments per PSUM window
NWIN = S // WIN                # 256 global windows
NWC = NWIN // NCORES           # 32 windows per core
EPS = 1e-10
SENT = 999.0                   # idxl sentinel for padded rows -> G row = 0

LAST_EXEC_NS = None
LAST_RESULTS = None

_module_cache = {}


def _build_module(tpw, has_bias, needs_eps):
    """tpw: tuple of NWC per-window tile counts (uniform across cores)."""
    key = (tpw, has_bias, needs_eps)
    if key in _module_cache:
        return _module_cache[key]

    import concourse.bass as bass  # noqa: F401
    import concourse.tile as tile
    from concourse import bacc, mybir

    f32 = mybir.dt.float32
    bf16 = mybir.dt.bfloat16
    ALU = mybir.AluOpType
    AX = mybir.AxisListType
    ACTF = mybir.ActivationFunctionType

    NT = int(sum(tpw))
    tbase = [0]
    for t in tpw:
        tbase.append(tbase[-1] + t)

    nc = bacc.Bacc(
        "TRN2",
        target_bir_lowering=False,
        debug=False,
        enable_asserts=True,
        num_devices=NCORES,
    )

    # mask split: u8 windows are converted to bf16 on the (slack) ACT
    # engine; bf16 windows DMA directly.  Balances DMA vs ACT occupancy.
    u8_wins = tuple(w for w in range(NWC) if w % 2 == 0)
    bf_wins = tuple(w for w in range(NWC) if w % 2 == 1)
    u8_cols = {}
    bf_cols = {}
    off = 0
    for w in u8_wins:
        u8_cols[w] = off
        off += tpw[w] * WIN
    n_u8_cols = off
    off = 0
    for w in bf_wins:
        bf_cols[w] = off
        off += tpw[w] * WIN
    n_bf_cols = off

    xp = nc.dram_tensor("xp", [P, NT * (D + 1)], bf16, kind="ExternalInput")
    masku8 = nc.dram_tensor("masku8", [P, n_u8_cols], mybir.dt.uint8,
                            kind="ExternalInput")
    maskbf = nc.dram_tensor("maskbf", [P, n_bf_cols], bf16,
                            kind="ExternalInput")
    wall = nc.dram_tensor("wall", [P, NT], f32, kind="ExternalInput")
    identb = nc.dram_tensor("identb", [WIN, WIN], f32, kind="ExternalInput")
    msgwp = nc.dram_tensor("msgwp", [D, D], f32, kind="ExternalInput")
    gatebrep = nc.dram_tensor("gatebrep", [P, 1], f32, kind="ExternalInput")
    prep = nc.dram_tensor("prep", [P, 1], f32, kind="ExternalInput")
    if has_bias:
        msgbrep = nc.dram_tensor("msgbrep", [P, D], f32, kind="ExternalInput")
    out = nc.dram_tensor("out", [NWC * WIN, D], f32, kind="ExternalOutput")

    with tile.TileContext(nc) as tc:
        from contextlib import ExitStack

        with ExitStack() as ctx:
            const_pool = ctx.enter_context(tc.tile_pool(name="const", bufs=1))
            xs_pool = ctx.enter_context(tc.tile_pool(name="xs", bufs=7))
            tr_pool = ctx.enter_context(tc.tile_pool(name="tr", bufs=4))
            lg_pool = ctx.enter_context(tc.tile_pool(name="lg", bufs=5))
            g_pool = ctx.enter_context(tc.tile_pool(name="gm", bufs=4))
            psA_pool = ctx.enter_context(tc.tile_pool(name="psA", bufs=3, space="PSUM"))
            psT_pool = ctx.enter_context(tc.tile_pool(name="psT", bufs=2, space="PSUM"))
            ps2_pool = ctx.enter_context(tc.tile_pool(name="ps2", bufs=2, space="PSUM"))
            ph_pool = ctx.enter_context(tc.tile_pool(name="ph", bufs=3))

            ident_t = const_pool.tile([WIN, WIN], f32)
            nc.sync.dma_start(ident_t[:], identb[:, :])
            msgw_t = const_pool.tile([D, D], f32)
            nc.sync.dma_start(msgw_t[:], msgwp[:, :])
            gateb_t = const_pool.tile([P, 1], f32)
            nc.sync.dma_start(gateb_t[:], gatebrep[:, :])
            p_t = const_pool.tile([P, 1], f32)
            nc.sync.dma_start(p_t[:], prep[:, :])
            if has_bias:
                msgb_t = const_pool.tile([P, D], f32)
                nc.sync.dma_start(msgb_t[:], msgbrep[:, :])

            # hoisted: plw = pow_p * ln(w) for every tile in two ops
            w_t = const_pool.tile([P, NT], f32)
            nc.sync.dma_start(w_t[:], wall[:, :])
            plw_t = const_pool.tile([P, NT], f32)
            nc.scalar.activation(out=plw_t[:], in_=w_t[:], func=ACTF.Ln)
            nc.vector.tensor_scalar_mul(plw_t[:], plw_t[:], p_t[:, 0:1])

            chains = {}

            fetched = {}

            def emit_fetch(w):
                T = tpw[w]
                base = tbase[w]
                xs = xs_pool.tile([P, T * (D + 1)], bf16, tag="xs", name=f"xs{w}")
                nc.sync.dma_start(
                    xs[:], xp[:, base * (D + 1) : (base + T) * (D + 1)]
                )
                if w in u8_cols:
                    mku = xs_pool.tile([P, T * WIN], mybir.dt.uint8, tag="mku",
                                       name=f"mu{w}")
                    c0 = u8_cols[w]
                    nc.sync.dma_start(mku[:], masku8[:, c0 : c0 + T * WIN])
                else:
                    mku = xs_pool.tile([P, T * WIN], bf16, tag="mk",
                                       name=f"mk{w}")
                    c0 = bf_cols[w]
                    nc.sync.dma_start(mku[:], maskbf[:, c0 : c0 + T * WIN])
                fetched[w] = (xs, mku)

            def emit_logits(w):
                T = tpw[w]
                base = tbase[w]
                xs, mku = fetched.pop(w)
                xs3 = xs[:].rearrange("p (t c) -> p t c", c=D + 1)
                if w in u8_cols:
                    mk = g_pool.tile([P, T * WIN], bf16, tag="mkb",
                                     name=f"mb{w}")
                    nc.scalar.activation(out=mk[:], in_=mku[:], func=ACTF.Copy)
                else:
                    mk = mku
                rA = tr_pool.tile([P, T * 64], bf16, tag="rA", name=f"rA{w}")
                rA3 = rA[:].rearrange("p (t c) -> p t c", c=64)
                rB = tr_pool.tile([P, T * 32], bf16, tag="rB", name=f"rB{w}")
                rB3 = rB[:].rearrange("p (t c) -> p t c", c=32)
                rC = tr_pool.tile([P, T * 16], bf16, tag="rC", name=f"rC{w}")
                rC3 = rC[:].rearrange("p (t c) -> p t c", c=16)
                TT = nc.vector.tensor_tensor
                TT(out=rA3, in0=xs3[:, :, 0:64], in1=xs3[:, :, 64:128], op=ALU.add)
                TT(out=rB3, in0=rA3[:, :, 0:32], in1=rA3[:, :, 32:64], op=ALU.add)
                TT(out=rC3, in0=rB3[:, :, 0:16], in1=rB3[:, :, 16:32], op=ALU.add)
                lg = lg_pool.tile([P, T], f32, tag="lg", name=f"lg{w}")
                nc.vector.reduce_sum(out=lg[:], in_=rC3, axis=AX.X)
                lg2 = lg_pool.tile([P, T], f32, tag="lg2", name=f"lh{w}")
                nc.vector.tensor_add(lg2[:], lg[:], plw_t[:, base : base + T])
                # exp broadcasts straight into a [p, t, s] strip (ACT reads
                # the stride-0 view); the strip is bf16 + innermost-packed so
                # the G multiply below runs in DVE 2x mode on a contiguous
                # per-tile layout (fast LDWEIGHTS for the matmul).
                gexs = g_pool.tile([P, T * WIN], bf16, tag="gexs", name=f"ge{w}")
                gexs3 = gexs[:].rearrange("p (t s) -> p t s", s=WIN)
                lg2b = lg2[:].unsqueeze(2).to_broadcast((P, T, WIN))
                nc.scalar.activation(
                    out=gexs3, in_=lg2b, func=ACTF.Exp, bias=gateb_t[:, 0:1]
                )
                chains[w] = (xs3, mk, gexs)

            gbuilt = {}

            def emit_gbuild(w):
                T = tpw[w]
                xs3, mk, gexs = chains.pop(w)
                Gs = g_pool.tile([P, T * WIN], bf16, tag="G", name=f"G{w}")
                G3 = Gs[:].rearrange("p (t s) -> p t s", s=WIN)
                nc.vector.tensor_tensor(out=Gs[:], in0=mk[:], in1=gexs[:],
                                        op=ALU.mult)
                gbuilt[w] = (xs3, G3)

            def emit_matmuls(w, psA):
                T = tpw[w]
                xs3, G3 = gbuilt.pop(w)
                for j in range(T):
                    nc.tensor.matmul(
                        out=psA[:],
                        lhsT=G3[:, j, :],
                        rhs=xs3[:, j, :],
                        start=(j == 0),
                        stop=(j == T - 1),
                    )

            def emit_phase2(w, psA):
                sbA = ph_pool.tile([WIN, D + 1], f32, tag="sbA", name=f"sbA{w}")
                nc.scalar.activation(out=sbA[:], in_=psA[:], func=ACTF.Copy)
                rcp = ph_pool.tile([WIN, 1], f32, tag="rcp", name=f"rc{w}")
                if needs_eps:
                    deno = ph_pool.tile([WIN, 1], f32, tag="deno", name=f"dn{w}")
                    nc.scalar.activation(
                        out=deno[:], in_=psA[:, D : D + 1], func=ACTF.Copy,
                        bias=EPS,
                    )
                    nc.vector.reciprocal(out=rcp[:], in_=deno[:])
                else:
                    nc.vector.reciprocal(out=rcp[:], in_=sbA[:, D : D + 1])
                psAT = psT_pool.tile([P, WIN], f32, tag="AT", name=f"AT{w}")
                nc.tensor.transpose(
                    out=psAT[:], in_=sbA[:, 0:D], identity=ident_t[:, :]
                )
                sbAT = ph_pool.tile([P, WIN], f32, tag="sbAT", name=f"sT{w}")
                nc.scalar.activation(out=sbAT[:], in_=psAT[:], func=ACTF.Copy)
                ps2 = ps2_pool.tile([WIN, D], f32, tag="out2", name=f"o2{w}")
                nc.tensor.matmul(
                    out=ps2[:], lhsT=sbAT[:], rhs=msgw_t[:], start=True, stop=True
                )
                outsb = ph_pool.tile([WIN, D], f32, tag="outsb", name=f"ou{w}")
                nc.scalar.activation(
                    out=outsb[:], in_=ps2[:], func=ACTF.Copy, scale=rcp[:, 0:1]
                )
                fin = outsb
                if has_bias:
                    coef = ph_pool.tile([WIN, 1], f32, tag="coef", name=f"cf{w}")
                    nc.vector.tensor_tensor(
                        out=coef[:], in0=psA[:, D : D + 1], in1=rcp[:], op=ALU.mult
                    )
                    bt = ph_pool.tile([WIN, D], f32, tag="bt", name=f"bt{w}")
                    nc.scalar.activation(
                        out=bt[:], in_=msgb_t[:WIN, :], func=ACTF.Copy,
                        scale=coef[:, 0:1],
                    )
                    fin = ph_pool.tile([WIN, D], f32, tag="fin", name=f"fi{w}")
                    nc.vector.tensor_add(fin[:], outsb[:], bt[:])
                nc.sync.dma_start(out[w * WIN : (w + 1) * WIN, :], fin[:])

            # software pipeline, stages per loop iteration w:
            #   DMA: fetch(w+4)      -- a full block of lead before its tree
            #   PE:  matmuls(w)      -- G(w) was built in iteration w-1, so
            #                           PE is fed at block start & stays
            #                           ramped (full p-state)
            #   DVE: G-TT(w+1), recip(w-1), tree(w+3)
            #   ACT: phase2 copies(w-1) first (ready), then exp(w+3) which
            #        waits on tree(w+3) -- no head-of-line blocking
            for w_ in range(4):
                emit_fetch(w_)
            emit_logits(0)
            emit_logits(1)
            emit_logits(2)
            emit_gbuild(0)
            prev = None
            for w in range(NWC):
                psA = psA_pool.tile([WIN, D + 1], f32, tag="psA", name=f"psA{w}")
                emit_matmuls(w, psA)
                if w + 1 < NWC:
                    emit_gbuild(w + 1)
                if w + 4 < NWC:
                    emit_fetch(w + 4)
                if prev is not None:
                    emit_phase2(*prev)
                if w + 3 < NWC:
                    emit_logits(w + 3)
                prev = (w, psA)
            emit_phase2(*prev)

    nc.compile()
    _module_cache[key] = nc
    return nc


def _layout(idx):
    bounds = np.searchsorted(idx, np.arange(0, S + 1, WIN)).astype(np.int64)
    counts = np.diff(bounds)
    tiles = -(-counts // P)
    tpw = tiles.reshape(NCORES, NWC).max(axis=0)
    tbase = np.concatenate([[0], np.cumsum(tpw)]).astype(np.int64)
    return bounds, counts, tpw, tbase


def _shard_inputs(x, idx, w, gwc, bounds, counts, tpw, tbase):
    """Pad + reorder host arrays into the per-core device layouts."""
    from concourse import mybir

    bf16 = mybir.dt.np(mybir.dt.bfloat16)
    n = idx.shape[0]
    NT = int(tbase[-1])

    wg = np.repeat(np.arange(NWIN, dtype=np.int64), counts)
    k = np.arange(n, dtype=np.int64) - np.repeat(bounds[:-1], counts)
    w_slot = wg % NWC
    core = wg // NWC
    flat = (tbase[w_slot] + k // P) * P + (k % P)
    rowpos = core * (NT * P) + flat

    xall = np.zeros((NCORES * NT * P, D + 1), dtype=np.float32)
    xall[rowpos, 0:D] = x * gwc[None, :]
    xall[rowpos, D] = 1.0

    mask = np.zeros((NCORES * NT * P, WIN), dtype=np.uint8)
    mask[rowpos, (idx - wg * WIN).astype(np.int64)] = 1
    wpad = np.ones(NCORES * NT * P, dtype=np.float32)
    wpad[rowpos] = w

    # device layout per core: [P, NT*(D+1)] bf16, tile-major columns.
    # mask is bf16 one-hot [p, tile, seg] (2-byte so DVE 2x mode holds).
    xdev, maskdev = [], []
    for c in range(NCORES):
        xc = xall[c * NT * P : (c + 1) * NT * P].reshape(NT, P, D + 1)
        xdev.append(
            np.ascontiguousarray(xc.transpose(1, 0, 2)).reshape(P, NT * (D + 1))
            .astype(bf16)
        )
        mc = mask[c * NT * P : (c + 1) * NT * P].reshape(NT, P, WIN)
        md = np.ascontiguousarray(mc.transpose(1, 0, 2)).reshape(P, NT * WIN)
        u8_parts, bf_parts = [], []
        for w_ in range(NWC):
            blk = md[:, int(tbase[w_]) * WIN : int(tbase[w_ + 1]) * WIN]
            (u8_parts if w_ % 2 == 0 else bf_parts).append(blk)
        maskdev.append((
            np.ascontiguousarray(np.concatenate(u8_parts, axis=1)),
            np.ascontiguousarray(np.concatenate(bf_parts, axis=1)).astype(bf16),
        ))
    wdev = np.ascontiguousarray(wpad.reshape(NCORES, NT, P).transpose(0, 2, 1))
    return xdev, maskdev, wdev


def _ensure_ntff_hook():
    """The image's antenv package lacks axon_hooks; shim it so trace=True
    can register the ctypes NTFF hook from trn_agent_boot."""
    try:
        from antenv.axon_hooks import get_axon_ntff_profile_hook  # noqa: F401

        return True
    except ImportError:
        pass
    try:
        import types

        import antenv
        from trn_agent_boot.trn_boot import _ntff_profile_via_ctypes

        mod = types.ModuleType("antenv.axon_hooks")
        _hook = [None]
        mod.set_axon_ntff_profile_hook = lambda h: _hook.__setitem__(0, h)
        mod.get_axon_ntff_profile_hook = lambda: _hook[0]
        sys.modules["antenv.axon_hooks"] = mod
        antenv.axon_hooks = mod
        mod.set_axon_ntff_profile_hook(
            _ntff_profile_via_ctypes("/opt/axon/libaxon_pjrt.so")
        )
        return True
    except Exception as e:  # degrade to untraced run
        print(f"ntff hook install failed: {type(e).__name__}: {e}")
        return False


def kernel(x, index, weights, gate_w, gate_b, msg_w, msg_b, pow_p):
    global LAST_EXEC_NS, LAST_RESULTS
    from concourse import mybir

    bf16 = mybir.dt.np(mybir.dt.bfloat16)

    x = np.ascontiguousarray(np.asarray(x, dtype=np.float32))
    idx = np.asarray(index).astype(np.int64).ravel()
    w = np.asarray(weights, dtype=np.float32).ravel()
    gate_w = np.asarray(gate_w, dtype=np.float32).reshape(D)
    gate_b = np.asarray(gate_b, dtype=np.float32).reshape(1)
    msg_w = np.ascontiguousarray(np.asarray(msg_w, dtype=np.float32))
    msg_b = np.asarray(msg_b, dtype=np.float32).reshape(D)
    pow_p = np.asarray(pow_p, dtype=np.float32).reshape(1)

    if not np.all(idx[1:] >= idx[:-1]):
        perm = np.argsort(idx, kind="stable")
        idx = idx[perm]
        x = x[perm]
        w = w[perm]

    # fold gate_w into x; un-fold via msg_w' = msg_w / gwc (exact identity)
    gwc = np.where(np.abs(gate_w) < 1e-6,
                   np.where(gate_w < 0, -1e-6, 1e-6), gate_w).astype(np.float32)
    msgwp = (msg_w / gwc[:, None]).astype(np.float32)

    bounds, counts, tpw, tbase = _layout(idx)
    has_bias = bool(np.any(msg_b != 0.0))
    seg_counts = np.bincount(idx, minlength=S)
    needs_eps = bool(seg_counts.min() == 0)
    nc = _build_module(tuple(int(t) for t in tpw), has_bias, needs_eps)

    xdev, maskdev, wdev = _shard_inputs(x, idx, w, gwc, bounds, counts, tpw, tbase)

    identb = np.eye(WIN, dtype=np.float32)
    gatebrep = np.full((P, 1), gate_b[0], dtype=np.float32)
    prep = np.full((P, 1), pow_p[0], dtype=np.float32)

    from concourse.bass_utils import run_bass_kernel_spmd

    in_maps = []
    for c in range(NCORES):
        m = {
            "xp": xdev[c],
            "masku8": maskdev[c][0],
            "maskbf": maskdev[c][1],
            "wall": wdev[c],
            "identb": identb,
            "msgwp": msgwp,
            "gatebrep": gatebrep,
            "prep": prep,
        }
        if has_bias:
            m["msgbrep"] = np.tile(msg_b[None, :], (P, 1)).astype(np.float32)
        in_maps.append(m)

    trace = bool(os.environ.get("KERNEL_TRACE"))
    if trace:
        trace = _ensure_ntff_hook()
    res = run_bass_kernel_spmd(
        nc, in_maps, core_ids=list(range(NCORES)), trace=trace
    )
    LAST_RESULTS = res
    LAST_EXEC_NS = res.exec_time_ns

    out = np.concatenate([res.results[c]["out"] for c in range(NCORES)], axis=0)
    return out.astype(np.float32)


def kernel_numpy(x, index, weights, gate_w, gate_b, msg_w, msg_b, pow_p):
    """Host-side mirror of the device algorithm with bf16 rounding (debug)."""
    try:
        import ml_dtypes
        bf16 = ml_dtypes.bfloat16
    except ImportError:
        from concourse import mybir
        bf16 = mybir.dt.np(mybir.dt.bfloat16)

    x = np.asarray(x, dtype=np.float32)
    idx = np.asarray(index).astype(np.int64).ravel()
    w = np.asarray(weights, dtype=np.float32).ravel()
    gw = np.asarray(gate_w, dtype=np.float32).reshape(D)
    gwc = np.where(np.abs(gw) < 1e-6, np.where(gw < 0, -1e-6, 1e-6), gw)
    xg = (x * gwc[None, :]).astype(bf16)
    # binary-tree logit reduce: levels 1-2 bf16, rest f32 (mirrors device)
    t = xg.astype(np.float32)
    width = D
    lvl = 0
    while width > 1:
        width //= 2
        lvl += 1
        t = t[:, 0:width] + t[:, width : 2 * width]
        if lvl <= 3:
            t = t.astype(bf16).astype(np.float32)
    logit = t[:, 0]
    g = np.exp(
        logit
        + np.asarray(pow_p).reshape(1)[0] * np.log(w)
        + np.asarray(gate_b).reshape(1)[0]
    ).astype(np.float32)
    A = np.zeros((S, D), dtype=np.float64)
    den = np.zeros(S, dtype=np.float64)
    gb = g.astype(bf16).astype(np.float64)
    np.add.at(A, idx, gb[:, None] * xg.astype(np.float64))
    np.add.at(den, idx, gb)
    Ab = A.astype(np.float32).astype(np.float64)
    Wb = (np.asarray(msg_w, np.float32) / gwc[:, None]).astype(np.float64)
    out = (Ab @ Wb) / (den[:, None] + EPS)
    out = out + (den / (den + EPS))[:, None] * np.asarray(msg_b).reshape(1, D)
    return out.astype(np.float32)


# revision 50
# speedup vs baseline: 1.1980x; 1.1980x over previous
"""AttentionPooling (segment softmax-pool) Trainium2 kernel, v3.

out[s,:] = sum_n 1[idx[n]==s] * gnorm[n] * (x[n,:] @ msg_w + msg_b)
  gnorm[n] = w[n]^p * exp(gate[n]) / (denom[seg] + eps)   (max-sub skipped:
  mathematically identical after normalization, logits are O(5))

v3 restructure vs the f32 baseline (647us -> target <250us):
  * everything bf16: PE matmuls 1 cyc/row (vs 4 for f32), DVE 2x/4x modes,
    half the HBM traffic.  rel tolerance is 2e-2; bf16 lands ~1e-3.
  * gate_w folded into x on the host: device sees xg = x * gate_w and
    msg_w' = msg_w / gate_w (exact identity: A@msg_w == (A*gw)@(msg_w/gw),
    denominator column unscaled).  The per-row gate logit then needs NO
    multiply on device -- just a reduction, done as a 7-level binary add
    tree (tensor_tensor has a 2x mode; tensor_reduce has none).  Levels
    1-2 are bf16 (the bulk of the work), levels 3-7 f32: an all-bf16
    tree costs 1.1e-2 rel err (vs the 2e-2 gate), this mix 4.6e-3.
  * one-hot G built per WINDOW (not per tile): host supplies a u8 one-hot
    mask strip [128, T*64]; one DVE tensor_tensor multiply against a
    stride-0 broadcast of gex[p,t] builds the whole window's G strip.
    (per-tile tensor_scalar builds cost ~207ns/op of fixed overhead;
    per-window strips amortize it 30x.)
  * exact per-window tiling: module is specialized (and cached) to the
    actual per-window tile counts (max over cores per window slot),
    ~1019 tiles/core vs 1088 padded.

Phase 1 per 128-row tile t of window w (64 segments per window):
  G[p,s]    = (iota[s] == idxl[p,t]) * gex[p,t]          (DVE, 4x)
  psA[s,c] += sum_p G[p,s] * xg[p,t,c]                   (PE, c = 0..128,
              col 128 is the ones column -> denominators)
Phase 2 per window: numer = (psA[:,0:128])^T-major matmul with msg_w',
  out = numer * (1/(denom+eps)) (+ msg_b term only if msg_b != 0).
"""

import os
import sys
import numpy as np

for _p in ("/opt/trn_rl_repo", "/root/.axon_site/_ro/trn_rl_repo"):
    if os.path.isdir(_p) and _p not in sys.path:
        sys.path.insert(0, _p)

P = 128
S = 16384
D = 128
NCORES = 8
WIN = 64                       # segments per PSUM window
NWIN = S // WIN                # 256 global windows
NWC = NWIN // NCORES           # 32 windows per core
EPS = 1e-10
SENT = 999.0                   # idxl sentinel for padded rows -> G row = 0

LAST_EXEC_NS = None
LAST_RESULTS = None

_module_cache = {}


def _build_module(tpw, has_bias, needs_eps):
    """tpw: tuple of NWC per-window tile counts (uniform across cores)."""
    key = (tpw, has_bias, needs_eps)
    if key in _module_cache:
        return _module_cache[key]

    import concourse.bass as bass  # noqa: F401
    import concourse.tile as tile
    from concourse import bacc, mybir

    f32 = mybir.dt.float32
    bf16 = mybir.dt.bfloat16
    ALU = mybir.AluOpType
    AX = mybir.AxisListType
    ACTF = mybir.ActivationFunctionType

    NT = int(sum(tpw))
    tbase = [0]
    for t in tpw:
        tbase.append(tbase[-1] + t)

    nc = bacc.Bacc(
        "TRN2",
        target_bir_lowering=False,
        debug=False,
        enable_asserts=True,
        num_devices=NCORES,
    )

    # mask split: u8 windows are converted to bf16 on the (slack) ACT
    # engine; bf16 windows DMA directly.  Balances DMA vs ACT occupancy.
    u8_wins = tuple(w for w in range(NWC) if w % 8 < 5)
    bf_wins = tuple(w for w in range(NWC) if w % 8 >= 5)
    u8_cols = {}
    bf_cols = {}
    off = 0
    for w in u8_wins:
        u8_cols[w] = off
        off += tpw[w] * WIN
    n_u8_cols = off
    off = 0
    for w in bf_wins:
        bf_cols[w] = off
        off += tpw[w] * WIN
    n_bf_cols = off

    xp = nc.dram_tensor("xp", [P, NT * (D + 1)], bf16, kind="ExternalInput")
    masku8 = nc.dram_tensor("masku8", [P, n_u8_cols], mybir.dt.uint8,
                            kind="ExternalInput")
    maskbf = nc.dram_tensor("maskbf", [P, n_bf_cols], bf16,
                            kind="ExternalInput")
    wall = nc.dram_tensor("wall", [P, NT], f32, kind="ExternalInput")
    identb = nc.dram_tensor("identb", [WIN, WIN], f32, kind="ExternalInput")
    msgwp = nc.dram_tensor("msgwp", [D, D], f32, kind="ExternalInput")
    gatebrep = nc.dram_tensor("gatebrep", [P, 1], f32, kind="ExternalInput")
    prep = nc.dram_tensor("prep", [P, 1], f32, kind="ExternalInput")
    if has_bias:
        msgbrep = nc.dram_tensor("msgbrep", [P, D], f32, kind="ExternalInput")
    out = nc.dram_tensor("out", [NWC * WIN, D], f32, kind="ExternalOutput")

    with tile.TileContext(nc) as tc:
        from contextlib import ExitStack

        with ExitStack() as ctx:
            const_pool = ctx.enter_context(tc.tile_pool(name="const", bufs=1))
            xs_pool = ctx.enter_context(tc.tile_pool(name="xs", bufs=6))
            tr_pool = ctx.enter_context(tc.tile_pool(name="tr", bufs=4))
            lg_pool = ctx.enter_context(tc.tile_pool(name="lg", bufs=5))
            g_pool = ctx.enter_context(tc.tile_pool(name="gm", bufs=4))
            psA_pool = ctx.enter_context(tc.tile_pool(name="psA", bufs=3, space="PSUM"))
            psT_pool = ctx.enter_context(tc.tile_pool(name="psT", bufs=2, space="PSUM"))
            ps2_pool = ctx.enter_context(tc.tile_pool(name="ps2", bufs=2, space="PSUM"))
            ph_pool = ctx.enter_context(tc.tile_pool(name="ph", bufs=3))

            ident_t = const_pool.tile([WIN, WIN], f32)
            nc.sync.dma_start(ident_t[:], identb[:, :])
            msgw_t = const_pool.tile([D, D], f32)
            nc.sync.dma_start(msgw_t[:], msgwp[:, :])
            gateb_t = const_pool.tile([P, 1], f32)
            nc.sync.dma_start(gateb_t[:], gatebrep[:, :])
            p_t = const_pool.tile([P, 1], f32)
            nc.sync.dma_start(p_t[:], prep[:, :])
            if has_bias:
                msgb_t = const_pool.tile([P, D], f32)
                nc.sync.dma_start(msgb_t[:], msgbrep[:, :])

            # hoisted: plw = pow_p * ln(w) for every tile in two ops
            w_t = const_pool.tile([P, NT], f32)
            nc.sync.dma_start(w_t[:], wall[:, :])
            plw_t = const_pool.tile([P, NT], f32)
            nc.scalar.activation(out=plw_t[:], in_=w_t[:], func=ACTF.Ln)
            nc.vector.tensor_scalar_mul(plw_t[:], plw_t[:], p_t[:, 0:1])

            chains = {}

            fetched = {}

            def emit_fetch(w):
                T = tpw[w]
                base = tbase[w]
                xs = xs_pool.tile([P, T * (D + 1)], bf16, tag="xs", name=f"xs{w}")
                nc.sync.dma_start(
                    xs[:], xp[:, base * (D + 1) : (base + T) * (D + 1)]
                )
                if w in u8_cols:
                    mku = xs_pool.tile([P, T * WIN], mybir.dt.uint8, tag="mku",
                                       name=f"mu{w}")
                    c0 = u8_cols[w]
                    nc.sync.dma_start(mku[:], masku8[:, c0 : c0 + T * WIN])
                else:
                    mku = xs_pool.tile([P, T * WIN], bf16, tag="mk",
                                       name=f"mk{w}")
                    c0 = bf_cols[w]
                    nc.sync.dma_start(mku[:], maskbf[:, c0 : c0 + T * WIN])
                fetched[w] = (xs, mku)

            def emit_logits(w):
                T = tpw[w]
                base = tbase[w]
                xs, mku = fetched.pop(w)
                xs3 = xs[:].rearrange("p (t c) -> p t c", c=D + 1)
                if w in u8_cols:
                    mk = g_pool.tile([P, T * WIN], bf16, tag="mkb",
                                     name=f"mb{w}")
                    nc.scalar.activation(out=mk[:], in_=mku[:], func=ACTF.Copy)
                else:
                    mk = mku
                rA = tr_pool.tile([P, T * 64], bf16, tag="rA", name=f"rA{w}")
                rA3 = rA[:].rearrange("p (t c) -> p t c", c=64)
                rB = tr_pool.tile([P, T * 32], bf16, tag="rB", name=f"rB{w}")
                rB3 = rB[:].rearrange("p (t c) -> p t c", c=32)
                rC = tr_pool.tile([P, T * 16], bf16, tag="rC", name=f"rC{w}")
                rC3 = rC[:].rearrange("p (t c) -> p t c", c=16)
                TT = nc.vector.tensor_tensor
                TT(out=rA3, in0=xs3[:, :, 0:64], in1=xs3[:, :, 64:128], op=ALU.add)
                TT(out=rB3, in0=rA3[:, :, 0:32], in1=rA3[:, :, 32:64], op=ALU.add)
                TT(out=rC3, in0=rB3[:, :, 0:16], in1=rB3[:, :, 16:32], op=ALU.add)
                lg = lg_pool.tile([P, T], f32, tag="lg", name=f"lg{w}")
                nc.vector.reduce_sum(out=lg[:], in_=rC3, axis=AX.X)
                lg2 = lg_pool.tile([P, T], f32, tag="lg2", name=f"lh{w}")
                nc.vector.tensor_add(lg2[:], lg[:], plw_t[:, base : base + T])
                # exp broadcasts straight into a [p, t, s] strip (ACT reads
                # the stride-0 view); the strip is bf16 + innermost-packed so
                # the G multiply below runs in DVE 2x mode on a contiguous
                # per-tile layout (fast LDWEIGHTS for the matmul).
                gexs = g_pool.tile([P, T * WIN], bf16, tag="gexs", name=f"ge{w}")
                gexs3 = gexs[:].rearrange("p (t s) -> p t s", s=WIN)
                lg2b = lg2[:].unsqueeze(2).to_broadcast((P, T, WIN))
                nc.scalar.activation(
                    out=gexs3, in_=lg2b, func=ACTF.Exp, bias=gateb_t[:, 0:1]
                )
                chains[w] = (xs3, mk, gexs)

            gbuilt = {}

            def emit_gbuild(w):
                T = tpw[w]
                xs3, mk, gexs = chains.pop(w)
                Gs = g_pool.tile([P, T * WIN], bf16, tag="G", name=f"G{w}")
                G3 = Gs[:].rearrange("p (t s) -> p t s", s=WIN)
                nc.vector.tensor_tensor(out=Gs[:], in0=mk[:], in1=gexs[:],
                                        op=ALU.mult)
                gbuilt[w] = (xs3, G3)

            def emit_matmuls(w, psA):
                T = tpw[w]
                xs3, G3 = gbuilt.pop(w)
                for j in range(T):
                    nc.tensor.matmul(
                        out=psA[:],
                        lhsT=G3[:, j, :],
                        rhs=xs3[:, j, :],
                        start=(j == 0),
                        stop=(j == T - 1),
                    )

            def emit_phase2(w, psA):
                sbA = ph_pool.tile([WIN, D + 1], f32, tag="sbA", name=f"sbA{w}")
                nc.scalar.activation(out=sbA[:], in_=psA[:], func=ACTF.Copy)
                rcp = ph_pool.tile([WIN, 1], f32, tag="rcp", name=f"rc{w}")
                if needs_eps:
                    deno = ph_pool.tile([WIN, 1], f32, tag="deno", name=f"dn{w}")
                    nc.scalar.activation(
                        out=deno[:], in_=psA[:, D : D + 1], func=ACTF.Copy,
                        bias=EPS,
                    )
                    nc.vector.reciprocal(out=rcp[:], in_=deno[:])
                else:
                    nc.vector.reciprocal(out=rcp[:], in_=sbA[:, D : D + 1])
                psAT = psT_pool.tile([P, WIN], f32, tag="AT", name=f"AT{w}")
                nc.tensor.transpose(
                    out=psAT[:], in_=sbA[:, 0:D], identity=ident_t[:, :]
                )
                sbAT = ph_pool.tile([P, WIN], f32, tag="sbAT", name=f"sT{w}")
                nc.scalar.activation(out=sbAT[:], in_=psAT[:], func=ACTF.Copy)
                ps2 = ps2_pool.tile([WIN, D], f32, tag="out2", name=f"o2{w}")
                nc.tensor.matmul(
                    out=ps2[:], lhsT=sbAT[:], rhs=msgw_t[:], start=True, stop=True
                )
                outsb = ph_pool.tile([WIN, D], f32, tag="outsb", name=f"ou{w}")
                nc.scalar.activation(
                    out=outsb[:], in_=ps2[:], func=ACTF.Copy, scale=rcp[:, 0:1]
                )
                fin = outsb
                if has_bias:
                    coef = ph_pool.tile([WIN, 1], f32, tag="coef", name=f"cf{w}")
                    nc.vector.tensor_tensor(
                        out=coef[:], in0=psA[:, D : D + 1], in1=rcp[:], op=ALU.mult
                    )
                    bt = ph_pool.tile([WIN, D], f32, tag="bt", name=f"bt{w}")
                    nc.scalar.activation(
                        out=bt[:], in_=msgb_t[:WIN, :], func=ACTF.Copy,
                        scale=coef[:, 0:1],
                    )
                    fin = ph_pool.tile([WIN, D], f32, tag="fin", name=f"fi{w}")
                    nc.vector.tensor_add(fin[:], outsb[:], bt[:])
                nc.sync.dma_start(out[w * WIN : (w + 1) * WIN, :], fin[:])

            # software pipeline, stages per loop iteration w:
            #   DMA: fetch(w+4)      -- a full block of lead before its tree
            #   PE:  matmuls(w)      -- G(w) was built in iteration w-1, so
            #                           PE is fed at block start & stays
            #                           ramped (full p-state)
            #   DVE: G-TT(w+1), recip(w-1), tree(w+3)
            #   ACT: phase2 copies(w-1) first (ready), then exp(w+3) which
            #        waits on tree(w+3) -- no head-of-line blocking
            for w_ in range(3):
                emit_fetch(w_)
            emit_logits(0)
            emit_logits(1)
            emit_logits(2)
            emit_gbuild(0)
            prev = None
            for w in range(NWC):
                psA = psA_pool.tile([WIN, D + 1], f32, tag="psA", name=f"psA{w}")
                emit_matmuls(w, psA)
                if w + 1 < NWC:
                    emit_gbuild(w + 1)
                if w + 3 < NWC:
                    emit_fetch(w + 3)
                if prev is not None:
                    emit_phase2(*prev)
                if w + 3 < NWC:
                    emit_logits(w + 3)
                prev = (w, psA)
            emit_phase2(*prev)

    nc.compile()
    _module_cache[key] = nc
    return nc


def _layout(idx):
    bounds = np.searchsorted(idx, np.arange(0, S + 1, WIN)).astype(np.int64)
    counts = np.diff(bounds)
    tiles = -(-counts // P)
    tpw = tiles.reshape(NCORES, NWC).max(axis=0)
    tbase = np.concatenate([[0], np.cumsum(tpw)]).astype(np.int64)
    return bounds, counts, tpw, tbase


def _shard_inputs(x, idx, w, gwc, bounds, counts, tpw, tbase):
    """Pad + reorder host arrays into the per-core device layouts."""
    from concourse import mybir

    bf16 = mybir.dt.np(mybir.dt.bfloat16)
    n = idx.shape[0]
    NT = int(tbase[-1])

    wg = np.repeat(np.arange(NWIN, dtype=np.int64), counts)
    k = np.arange(n, dtype=np.int64) - np.repeat(bounds[:-1], counts)
    w_slot = wg % NWC
    core = wg // NWC
    flat = (tbase[w_slot] + k // P) * P + (k % P)
    rowpos = core * (NT * P) + flat

    xall = np.zeros((NCORES * NT * P, D + 1), dtype=np.float32)
    xall[rowpos, 0:D] = x * gwc[None, :]
    xall[rowpos, D] = 1.0

    mask = np.zeros((NCORES * NT * P, WIN), dtype=np.uint8)
    mask[rowpos, (idx - wg * WIN).astype(np.int64)] = 1
    wpad = np.ones(NCORES * NT * P, dtype=np.float32)
    wpad[rowpos] = w

    # device layout per core: [P, NT*(D+1)] bf16, tile-major columns.
    # mask is bf16 one-hot [p, tile, seg] (2-byte so DVE 2x mode holds).
    xdev, maskdev = [], []
    for c in range(NCORES):
        xc = xall[c * NT * P : (c + 1) * NT * P].reshape(NT, P, D + 1)
        xdev.append(
            np.ascontiguousarray(xc.transpose(1, 0, 2)).reshape(P, NT * (D + 1))
            .astype(bf16)
        )
        mc = mask[c * NT * P : (c + 1) * NT * P].reshape(NT, P, WIN)
        md = np.ascontiguousarray(mc.transpose(1, 0, 2)).reshape(P, NT * WIN)
        u8_parts, bf_parts = [], []
        for w_ in range(NWC):
            blk = md[:, int(tbase[w_]) * WIN : int(tbase[w_ + 1]) * WIN]
            (u8_parts if w_ % 8 < 5 else bf_parts).append(blk)
        maskdev.append((
            np.ascontiguousarray(np.concatenate(u8_parts, axis=1)),
            np.ascontiguousarray(np.concatenate(bf_parts, axis=1)).astype(bf16),
        ))
    wdev = np.ascontiguousarray(wpad.reshape(NCORES, NT, P).transpose(0, 2, 1))
    return xdev, maskdev, wdev


def _ensure_ntff_hook():
    """The image's antenv package lacks axon_hooks; shim it so trace=True
    can register the ctypes NTFF hook from trn_agent_boot."""
    try:
        from antenv.axon_hooks import get_axon_ntff_profile_hook  # noqa: F401

        return True
    except ImportError:
        pass
    try:
        import types

        import antenv
        from trn_agent_boot.trn_boot import _ntff_profile_via_ctypes

        mod = types.ModuleType("antenv.axon_hooks")
        _hook = [None]
        mod.set_axon_ntff_profile_hook = lambda h: _hook.__setitem__(0, h)
        mod.get_axon_ntff_profile_hook = lambda: _hook[0]
        sys.modules["antenv.axon_hooks"] = mod
        antenv.axon_hooks = mod
        mod.set_axon_ntff_profile_hook(
            _ntff_profile_via_ctypes("/opt/axon/libaxon_pjrt.so")
        )
        return True
    except Exception as e:  # degrade to untraced run
        print(f"ntff hook install failed: {type(e).__name__}: {e}")
        return False


def kernel(x, index, weights, gate_w, gate_b, msg_w, msg_b, pow_p):
    global LAST_EXEC_NS, LAST_RESULTS
    from concourse import mybir

    bf16 = mybir.dt.np(mybir.dt.bfloat16)

    x = np.ascontiguousarray(np.asarray(x, dtype=np.float32))
    idx = np.asarray(index).astype(np.int64).ravel()
    w = np.asarray(weights, dtype=np.float32).ravel()
    gate_w = np.asarray(gate_w, dtype=np.float32).reshape(D)
    gate_b = np.asarray(gate_b, dtype=np.float32).reshape(1)
    msg_w = np.ascontiguousarray(np.asarray(msg_w, dtype=np.float32))
    msg_b = np.asarray(msg_b, dtype=np.float32).reshape(D)
    pow_p = np.asarray(pow_p, dtype=np.float32).reshape(1)

    if not np.all(idx[1:] >= idx[:-1]):
        perm = np.argsort(idx, kind="stable")
        idx = idx[perm]
        x = x[perm]
        w = w[perm]

    # fold gate_w into x; un-fold via msg_w' = msg_w / gwc (exact identity)
    gwc = np.where(np.abs(gate_w) < 1e-6,
                   np.where(gate_w < 0, -1e-6, 1e-6), gate_w).astype(np.float32)
    msgwp = (msg_w / gwc[:, None]).astype(np.float32)

    bounds, counts, tpw, tbase = _layout(idx)
    has_bias = bool(np.any(msg_b != 0.0))
    seg_counts = np.bincount(idx, minlength=S)
    needs_eps = bool(seg_counts.min() == 0)
    nc = _build_module(tuple(int(t) for t in tpw), has_bias, needs_eps)

    xdev, maskdev, wdev = _shard_inputs(x, idx, w, gwc, bounds, counts, tpw, tbase)

    identb = np.eye(WIN, dtype=np.float32)
    gatebrep = np.full((P, 1), gate_b[0], dtype=np.float32)
    prep = np.full((P, 1), pow_p[0], dtype=np.float32)

    from concourse.bass_utils import run_bass_kernel_spmd

    in_maps = []
    for c in range(NCORES):
        m = {
            "xp": xdev[c],
            "masku8": maskdev[c][0],
            "maskbf": maskdev[c][1],
            "wall": wdev[c],
            "identb": identb,
            "msgwp": msgwp,
            "gatebrep": gatebrep,
            "prep": prep,
        }
        if has_bias:
            m["msgbrep"] = np.tile(msg_b[None, :], (P, 1)).astype(np.float32)
        in_maps.append(m)

    trace = bool(os.environ.get("KERNEL_TRACE"))
    if trace:
        trace = _ensure_ntff_hook()
    res = run_bass_kernel_spmd(
        nc, in_maps, core_ids=list(range(NCORES)), trace=trace
    )
    LAST_RESULTS = res
    LAST_EXEC_NS = res.exec_time_ns

    out = np.concatenate([res.results[c]["out"] for c in range(NCORES)], axis=0)
    return out.astype(np.float32)


def kernel_numpy(x, index, weights, gate_w, gate_b, msg_w, msg_b, pow_p):
    """Host-side mirror of the device algorithm with bf16 rounding (debug)."""
    try:
        import ml_dtypes
        bf16 = ml_dtypes.bfloat16
    except ImportError:
        from concourse import mybir
        bf16 = mybir.dt.np(mybir.dt.bfloat16)

    x = np.asarray(x, dtype=np.float32)
    idx = np.asarray(index).astype(np.int64).ravel()
    w = np.asarray(weights, dtype=np.float32).ravel()
    gw = np.asarray(gate_w, dtype=np.float32).reshape(D)
    gwc = np.where(np.abs(gw) < 1e-6, np.where(gw < 0, -1e-6, 1e-6), gw)
    xg = (x * gwc[None, :]).astype(bf16)
    # binary-tree logit reduce: levels 1-2 bf16, rest f32 (mirrors device)
    t = xg.astype(np.float32)
    width = D
    lvl = 0
    while width > 1:
        width //= 2
        lvl += 1
        t = t[:, 0:width] + t[:, width : 2 * width]
        if lvl <= 3:
            t = t.astype(bf16).astype(np.float32)
    logit = t[:, 0]
    g = np.exp(
        logit
        + np.asarray(pow_p).reshape(1)[0] * np.log(w)
        + np.asarray(gate_b).reshape(1)[0]
    ).astype(np.float32)
    A = np.zeros((S, D), dtype=np.float64)
    den = np.zeros(S, dtype=np.float64)
    gb = g.astype(bf16).astype(np.float64)
    np.add.at(A, idx, gb[:, None] * xg.astype(np.float64))
    np.add.at(den, idx, gb)
    Ab = A.astype(np.float32).astype(np.float64)
    Wb = (np.asarray(msg_w, np.float32) / gwc[:, None]).astype(np.float64)
    out = (Ab @ Wb) / (den[:, None] + EPS)
    out = out + (den / (den + EPS))[:, None] * np.asarray(msg_b).reshape(1, D)
    return out.astype(np.float32)
